# revision 16
# baseline (speedup 1.0000x reference)
"""Multi-headed attention (B=4, S=2048, D=1024, H=16) on 8 trn2 NeuronCores.

Sharding: core c handles batch b=c//2, head-half hh=c%2 (heads hh*8..hh*8+7).
Per core:
  phase 1: K projection (feature-major [512, 2048]) and V projection
           (row-major [2048, 512] + ones column) -- dense PE streams.
  phase 2: per query-tile t (512 queries), per head-pair j:
           Q projection on demand (dense PE burst), scores_T via paired
           K=64 matmuls on PE row halves, exp on ACT (1/8 scale folded in),
           AV with ones-augmented V accumulating unnormalized X + row sums
           (software-pipelined one chunk behind scores).
           After the 4 pairs: batch reciprocal of the 8 row-sum vectors,
           per-head K=1 broadcast matmul + in-place normalize of X, then
           the output projection for this query tile (dense PE burst).
Host: out[b] = core(2b) + core(2b+1) + bo.
"""

import os

import numpy as np

import concourse.tile as tile
from concourse import bacc, mybir
from concourse.bass_utils import run_bass_kernel_spmd

B, S, D, H = 4, 2048, 1024, 16
HD = D // 2          # feature columns per core (8 heads * 64)
KC = D // 128        # 8 contraction chunks over model dim
FT = HD // 128       # 4 feature tiles (head pairs)
ST = S // 512        # 4 query tiles
RT = S // 128        # 16 row tiles / S_k chunks

f32 = mybir.dt.float32
f32r = mybir.dt.float32r
EXP = mybir.ActivationFunctionType.Exp

_CACHED_NC = None
_LAST_IN_MAPS = None

_DT = os.environ.get("KMM_DT", "bf16")
MM_DT = mybir.dt.bfloat16 if _DT == "bf16" else f32r
# SBUF is tighter with 4-byte f32r tiles
XQR_BUFS = 16 if _DT == "bf16" else 8
PTP_BUFS = 8 if _DT == "bf16" else 4
QT_BUFS = 3 if _DT == "bf16" else 2


def build_nc():
    nc = bacc.Bacc("TRN2", target_bir_lowering=False, debug=False)

    xq_d = nc.dram_tensor("xq", (D, S), f32, kind="ExternalInput")
    xk_d = nc.dram_tensor("xk", (D, S), f32, kind="ExternalInput")
    xv_d = nc.dram_tensor("xv", (D, S), f32, kind="ExternalInput")
    wq_d = nc.dram_tensor("wq", (D, HD), f32, kind="ExternalInput")
    wk_d = nc.dram_tensor("wk", (D, HD), f32, kind="ExternalInput")
    wv_d = nc.dram_tensor("wv", (D, HD), f32, kind="ExternalInput")
    wo_d = nc.dram_tensor("wo", (HD, D), f32, kind="ExternalInput")
    bqr_d = nc.dram_tensor("bqr", (128, FT), f32, kind="ExternalInput")
    bkr_d = nc.dram_tensor("bkr", (128, FT), f32, kind="ExternalInput")
    bv_d = nc.dram_tensor("bv", (1, HD), f32, kind="ExternalInput")
    o_d = nc.dram_tensor("o", (S, D), f32, kind="ExternalOutput")

    with tile.TileContext(nc) as tc:
        with (
            tc.tile_pool(name="cpool", bufs=1) as cpool,
            tc.tile_pool(name="big", bufs=1) as big,
            tc.tile_pool(name="wpool", bufs=1) as wpool,
        ):
            ones_f = cpool.tile([1, 512], f32, name="ones_f")
            nc.gpsimd.memset(ones_f[:], 1.0)
            ones = cpool.tile([1, 512], MM_DT, name="ones")
            nc.vector.tensor_copy(ones[:], ones_f[:])
            ones_r = cpool.tile([1, 64], f32r, name="ones_r")
            nc.vector.tensor_copy(ones_r[:], ones_f[0:1, 0:64])
            onecol_f = cpool.tile([128, 1], f32, name="onecol_f")
            nc.gpsimd.memset(onecol_f[:], 1.0)

            bqr_s = cpool.tile([128, FT], f32, name="bqr_s")
            nc.sync.dma_start(bqr_s[:], bqr_d[:])
            bkr_s = cpool.tile([128, FT], f32, name="bkr_s")
            nc.sync.dma_start(bkr_s[:], bkr_d[:])
            bv_f = cpool.tile([1, HD], f32, name="bv_f")
            nc.sync.dma_start(bv_f[:], bv_d[:])
            bv_r = cpool.tile([1, HD], MM_DT, name="bv_r")
            nc.vector.tensor_copy(bv_r[:], bv_f[:])

            K = big.tile([128, FT, S], MM_DT, name="Kfm")
            Vs = big.tile([128, RT, 8, 65], MM_DT, name="Vs")
            X = big.tile([128, FT, S], MM_DT, name="Xfm")
            wo_s = big.tile([128, FT, D], MM_DT, name="wo_s")
            nc.vector.tensor_copy(
                Vs[:, :, :, 64:65],
                onecol_f[:, 0:1].to_broadcast((128, RT, 8, 1)),
            )

            # weights: DMA fp32 staging (wk split per-chunk so K-proj
            # starts early; wq/wo on the gpsimd queue to overlap)
            with tc.tile_pool(name="wstg", bufs=2) as wstg:
                w_r = {}
                for nm, w_d, eng in (("wk", wk_d, nc.sync),
                                     ("wv", wv_d, nc.gpsimd),
                                     ("wq", wq_d, nc.gpsimd),
                                     ("wo", wo_d, nc.gpsimd)):
                    nchunk = FT if nm == "wo" else KC
                    wr = (wo_s if nm == "wo" else
                          wpool.tile([128, KC, HD], MM_DT, name=f"{nm}_r"))
                    wf = wstg.tile([128, nchunk, w_d.shape[1]], f32,
                                   tag="wstg", name=f"{nm}f")
                    src = w_d[:].rearrange("(k p) n -> p k n", p=128)
                    ceng = nc.vector if nm in ("wk", "wv") else nc.gpsimd
                    for kc in range(nchunk):
                        eng.dma_start(wf[:, kc, :], src[:, kc, :])
                        ceng.tensor_copy(wr[:, kc, :], wf[:, kc, :])
                    w_r[nm] = wr
                wq_s, wk_s, wv_s = w_r["wq"], w_r["wk"], w_r["wv"]

                # ---------------- phase 1: K and V projections ----------------
                with (
                    tc.tile_pool(name="xstage", bufs=8) as xstage,
                    tc.tile_pool(name="xrstage", bufs=8) as xrstage,
                    tc.tile_pool(name="pacc", bufs=4, space="PSUM") as pacc,
                ):
                    def load_x(x_d, c0, c1):
                        xf = xstage.tile([128, 512], f32, tag="xs", name="xsf")
                        nc.sync.dma_start(
                            xf[:], x_d[c0 * 128 : (c0 + 1) * 128,
                                       c1 * 512 : (c1 + 1) * 512])
                        xr = xrstage.tile([128, 512], MM_DT, tag="xr", name="xsr")
                        nc.vector.tensor_copy(xr[:], xf[:])
                        return xr

                    # K projection, feature-major
                    for t in range(ST):
                        psums = [
                            pacc.tile([128, 512], f32, tag="pacc", name="pk")
                            for _ in range(FT)
                        ]
                        for kc in range(KC):
                            xr = load_x(xk_d, kc, t)
                            for ft in range(FT):
                                nc.tensor.matmul(
                                    psums[ft][:],
                                    wk_s[:, kc, ft * 128 : (ft + 1) * 128],
                                    xr[:],
                                    start=(kc == 0),
                                    stop=(kc == KC - 1),
                                )
                        for ft in range(FT):
                            nc.vector.tensor_scalar_add(
                                K[:, ft, t * 512 : (t + 1) * 512],
                                psums[ft][:],
                                bkr_s[:, ft : ft + 1],
                            )

                    # V projection, row-major + bias via K=1 matmul
                    for g in range(4):
                        xts = [load_x(xv_d, kc, g) for kc in range(KC)]
                        for rr in range(4):
                            rt = g * 4 + rr
                            ps = pacc.tile([128, 512], f32, tag="pacc", name="pv")
                            for kc in range(KC):
                                nc.tensor.matmul(
                                    ps[:],
                                    xts[kc][:, rr * 128 : (rr + 1) * 128],
                                    wv_s[:, kc, :],
                                    start=(kc == 0),
                                    stop=False,
                                )
                            nc.tensor.matmul(
                                ps[:],
                                ones[0:1, 0:128],
                                bv_r[0:1, :],
                                start=False,
                                stop=True,
                            )
                            nc.vector.tensor_copy(
                                Vs[:, rt, :, 0:64],
                                ps[:].rearrange("p (h e) -> p h e", h=8),
                            )

            # ---------------- phase 2: attention + out projection ----------
            with (
                tc.tile_pool(name="psc", bufs=2, space="PSUM") as psc,
                tc.tile_pool(name="px", bufs=2, space="PSUM") as px,
                tc.tile_pool(name="pq", bufs=2, space="PSUM") as pq,
                tc.tile_pool(name="xq2", bufs=4) as xq2,
                tc.tile_pool(name="xqr", bufs=XQR_BUFS) as xqr,
                tc.tile_pool(name="qt", bufs=QT_BUFS) as qtp,
                tc.tile_pool(name="ptp", bufs=PTP_BUFS) as ptp,
                tc.tile_pool(name="nrm", bufs=4) as nrm,
                tc.tile_pool(name="rsp", bufs=2) as rsp,
                tc.tile_pool(name="ostage", bufs=4) as ostage,
                tc.tile_pool(name="rsd", bufs=2, space="DRAM") as rsd,
            ):
                def emit_normalize(j2, rsj, tsl2):
                    rrh = nrm.tile([128, 512], f32, tag="rr", name="rr")
                    nc.vector.reciprocal_approx_fast(rrh[:], rsj[:])
                    rd = rsd.tile([2, 512], f32, tag="rd", name="rd")
                    for hh in range(2):
                        nc.sync.dma_start(
                            rd[hh : hh + 1, :],
                            rrh[32 * hh : 32 * hh + 1, :])
                    for hh in range(2):
                        pb = 64 * hh
                        bcs = nrm.tile([128, 512], f32, tag="bcs", name="bcs")
                        nc.sync.dma_start(
                            bcs[pb : pb + 64, :],
                            rd[hh : hh + 1, :].to_broadcast((64, 512)))
                        nc.vector.tensor_mul(
                            X[pb : pb + 64, j2, tsl2],
                            X[pb : pb + 64, j2, tsl2],
                            bcs[pb : pb + 64, :],
                        )

                def emit_outproj(t2):
                    for r2 in range(4):
                        rt = t2 * 4 + r2
                        rsl = slice(rt * 128, (rt + 1) * 128)
                        for n in range(2):
                            ps = pq.tile([128, 512], f32, tag="pacc", name="pso")
                            for fc in range(FT):
                                nc.tensor.matmul(
                                    ps[:],
                                    X[:, fc, rsl],
                                    wo_s[:, fc, n * 512 : (n + 1) * 512],
                                    start=(fc == 0),
                                    stop=(fc == FT - 1),
                                )
                            ot = ostage.tile([128, 512], f32, tag="os", name="os")
                            nc.vector.tensor_copy(ot[:], ps[:])
                            nc.sync.dma_start(
                                o_d[rsl, n * 512 : (n + 1) * 512], ot[:])

                pending = None
                norm_pending = None
                for t in range(ST):
                    tsl = slice(t * 512, (t + 1) * 512)
                    # stage this tile's xq columns once (used by all 4 pairs)
                    xqts = []
                    for kc in range(KC):
                        xf = xq2.tile([128, 512], f32, tag="xq2", name="xqf")
                        nc.gpsimd.dma_start(
                            xf[:], xq_d[kc * 128 : (kc + 1) * 128, tsl])
                        xr = xqr.tile([128, 512], MM_DT, tag="xqr", name="xqr")
                        nc.vector.tensor_copy(xr[:], xf[:])
                        xqts.append(xr)

                    for j in range(FT):
                        # Q projection on demand (dense PE burst)
                        qp = pq.tile([128, 512], f32, tag="pacc", name="qp")
                        for kc in range(KC):
                            nc.tensor.matmul(
                                qp[:],
                                wq_s[:, kc, j * 128 : (j + 1) * 128],
                                xqts[kc][:],
                                start=(kc == 0),
                                stop=(kc == KC - 1),
                            )
                        Qt = qtp.tile([128, 512], MM_DT, tag="qt", name="qt")
                        nc.vector.tensor_scalar_add(
                            Qt[:], qp[:], bqr_s[:, j : j + 1])
                        if norm_pending is not None:
                            emit_normalize(*norm_pending)
                            norm_pending = None

                        # attention, AV pipelined one chunk behind scores
                        xpA = px.tile([65, 512], f32, tag="px", name="xpA")
                        xpB = px.tile([65, 512], f32, tag="px", name="xpB")
                        pend = []
                        for cc in range(8):
                            sA = psc.tile([128, 2, 512], f32, tag="sc", name="sA")
                            sB = psc.tile([128, 2, 512], f32, tag="sc", name="sB")
                            for hf in range(2):
                                kc = 2 * cc + hf
                                ksl = slice(kc * 128, (kc + 1) * 128)
                                nc.tensor.matmul(
                                    sA[:, hf, :], K[0:64, j, ksl], Qt[0:64, :],
                                    start=True, stop=True, tile_position=(0, 0),
                                )
                                nc.tensor.matmul(
                                    sB[:, hf, :], K[64:128, j, ksl],
                                    Qt[64:128, :],
                                    start=True, stop=True, tile_position=(64, 0),
                                )
                            pA = ptp.tile([128, 2, 512], MM_DT, tag="pt",
                                          name="pA")
                            nc.scalar.activation(pA[:], sA[:], EXP, scale=0.125)
                            pB = ptp.tile([128, 2, 512], MM_DT, tag="pt",
                                          name="pB")
                            nc.scalar.activation(pB[:], sB[:], EXP, scale=0.125)
                            pend.append((cc, pA, pB))
                            if len(pend) > 2:
                                pcc, ppA, ppB = pend.pop(0)
                                for hf in range(2):
                                    kc = 2 * pcc + hf
                                    nc.tensor.matmul(
                                        xpA[:], Vs[:, kc, 2 * j, :],
                                        ppA[:, hf, :],
                                        start=(kc == 0), stop=False,
                                    )
                                    nc.tensor.matmul(
                                        xpB[:], Vs[:, kc, 2 * j + 1, :],
                                        ppB[:, hf, :],
                                        start=(kc == 0), stop=False,
                                    )
                        for pcc, ppA, ppB in pend:
                            for hf in range(2):
                                kc = 2 * pcc + hf
                                nc.tensor.matmul(
                                    xpA[:], Vs[:, kc, 2 * j, :], ppA[:, hf, :],
                                    start=False, stop=(kc == RT - 1),
                                )
                                nc.tensor.matmul(
                                    xpB[:], Vs[:, kc, 2 * j + 1, :], ppB[:, hf, :],
                                    start=False, stop=(kc == RT - 1),
                                )

                        # drain: unnormalized X and row sums to SBUF
                        nc.vector.tensor_copy(X[0:64, j, tsl], xpA[0:64, :])
                        nc.vector.tensor_copy(X[64:128, j, tsl], xpB[0:64, :])
                        rsj = rsp.tile([128, 512], f32, tag="rs", name="rs")
                        nc.vector.tensor_copy(rsj[0:1, :], xpA[64:65, :])
                        nc.vector.tensor_copy(rsj[32:33, :], xpB[64:65, :])

                        norm_pending = (j, rsj, tsl)

                        # previous tile's out projection: emitted after this
                        # tile's first pair so its normalize chain overlapped
                        if j == 0 and pending is not None:
                            emit_outproj(pending)
                    if norm_pending is not None:
                        emit_normalize(*norm_pending)
                        norm_pending = None
                    pending = t
                emit_outproj(pending)

    nc.compile()
    return nc


def kernel(**inputs):
    global _CACHED_NC, _LAST_IN_MAPS
    if _CACHED_NC is None:
        _CACHED_NC = build_nc()
    nc = _CACHED_NC

    query = np.asarray(inputs["query"], dtype=np.float32)
    key = np.asarray(inputs["key"], dtype=np.float32)
    value = np.asarray(inputs["value"], dtype=np.float32)
    fc_w = np.asarray(inputs["fc_w"], dtype=np.float32)
    Wq = np.asarray(inputs["Wq"], dtype=np.float32)
    Wk = np.asarray(inputs["Wk"], dtype=np.float32)
    Wv = np.asarray(inputs["Wv"], dtype=np.float32)
    Wo = np.asarray(inputs["Wo"], dtype=np.float32)
    bq = np.asarray(inputs["bq"], dtype=np.float32)
    bk = np.asarray(inputs["bk"], dtype=np.float32)
    bv = np.asarray(inputs["bv"], dtype=np.float32)
    bo = np.asarray(inputs["bo"], dtype=np.float32)

    wq_eff = fc_w * Wq

    in_maps = []
    for c in range(8):
        b, hh = c // 2, c % 2
        hs = slice(hh * HD, (hh + 1) * HD)
        in_maps.append({
            "xq": np.ascontiguousarray(query[b].T),
            "xk": np.ascontiguousarray(key[b].T),
            "xv": np.ascontiguousarray(value[b].T),
            "wq": np.ascontiguousarray(wq_eff[:, hs]),
            "wk": np.ascontiguousarray(Wk[:, hs]),
            "wv": np.ascontiguousarray(Wv[:, hs]),
            "wo": np.ascontiguousarray(Wo[hs, :]),
            "bqr": np.ascontiguousarray(bq[hs].reshape(FT, 128).T),
            "bkr": np.ascontiguousarray(bk[hs].reshape(FT, 128).T),
            "bv": bv[None, hs],
        })

    _LAST_IN_MAPS = in_maps
    res = run_bass_kernel_spmd(nc, in_maps, core_ids=list(range(8)))

    out = np.empty((B, S, D), dtype=np.float32)
    for b in range(B):
        out[b] = res.results[2 * b]["o"] + res.results[2 * b + 1]["o"] + bo
    return out
